# revision 10
# baseline (speedup 1.0000x reference)
"""GRU cell (single timestep) on 8 TRN2 NeuronCores, data-parallel over batch.

Contract: kernel(**inputs) takes FULL numpy inputs (as produced by the
problem's setup_inputs()) and returns the FULL (16384, 1024) float32 output.

Strategy (fp8 DoubleRow, v4):
  - Shard batch (16384) across 8 cores -> 2048 rows/core. Replicate weights.
  - Feature-major layout: contraction dim on partitions, batch on free dim.
      a8    [128, 12, 2048] f8  k-chunks 0-3 = x (e4m3), 4-11 = hidden (e4m3)
      hb    [128, 8, 2048] bf16 hidden for elementwise
      xb    [128, 4, 2048] bf16 x for the bf16 hc x-side matmul
      wr8   [128, 12, 1024] f8  Wxr (4 chunks) ++ Whr (8 chunks), pre-scaled
      wz8   [128, 12, 1024] f8  Wxz ++ Whz, pre-scaled
      wxh16 [128, 4, 1024] bf16 Wxh pre-scaled; whh8 [128, 8, 1024] f8
      bias  [128, 24] f32; outT [128, 8, 2048] bf16
  - r/z gates fully fp8 e4m3 with perf_mode=DoubleRow (2 contraction rows
    per PE cell per cycle); hc gate: x-side bf16 (precision-critical),
    h-side fp8 DR. All weights pre-scaled by 2048 (power of two); the
    activation instruction divides it back out. e4m3 operand error budget
    measured: rel_fro ~1.5e-2 vs the f32 reference (gate 2e-2).
  - Block loop (4 x 512 batch cols) INSIDE the k loop so one loaded
    stationary weight serves 4 matmuls (DoubleRow disables fast weight
    load; LDWEIGHTS must hide under 4 matmuls).
  - All post-matmul elementwise in bf16 (2x DVE); output written bf16 and
    upcast on host.
"""

import sys

if "/opt/trn_rl_repo" not in sys.path:
    sys.path.insert(0, "/opt/trn_rl_repo")

import numpy as np
import ml_dtypes

import concourse.bass as bass
import concourse.tile as tile
from concourse import bacc, mybir
from concourse.bass_utils import run_bass_kernel_spmd

P = 128
NCORES = 8
BATCH = 16384
NB = BATCH // NCORES          # 2048 rows per core
IN = 512
HID = 1024
KX = IN // P                  # 4
KH = HID // P                 # 8
KA = KX + KH                  # 12 chunks in the packed a8 tensor
M = HID // P                  # 8 output-feature chunks
BLK = 512                     # batch columns per block
NBLK = NB // BLK              # 4

F32 = mybir.dt.float32
BF16 = mybir.dt.bfloat16
F8 = mybir.dt.float8e4
DR = mybir.MatmulPerfMode.DoubleRow

SW = 2048.0                   # weight pre-scale (power of two, exact)
INV_SW = 1.0 / SW

_CACHE = {}


def _fp8_gate(nc, psl, w_s, act, m, bsl, blk_outer=False):
    """All-fp8 gate: 6 DoubleRow k-pairs over the packed 12-chunk operands."""
    mo = bass.ts(m, P)
    NT = KA // 2
    if blk_outer:
        # k-contiguous per block: used for the very first group so matmuls
        # can start as soon as block 0's operands arrive.
        for b in range(NBLK):
            for t in range(NT):
                nc.tensor.matmul(
                    psl[b][:], w_s[:, 2 * t : 2 * t + 2, mo],
                    act[:, 2 * t : 2 * t + 2, bsl[b]],
                    start=(t == 0), stop=(t == NT - 1), perf_mode=DR,
                )
    else:
        for t in range(NT):
            for b in range(NBLK):
                nc.tensor.matmul(
                    psl[b][:], w_s[:, 2 * t : 2 * t + 2, mo],
                    act[:, 2 * t : 2 * t + 2, bsl[b]],
                    start=(t == 0), stop=(t == NT - 1), perf_mode=DR,
                )


def _hc_gate(nc, psl, wx_s, xb_s, wh_s, rh8_s, m, bsl):
    """hc gate: bf16 x-side (4 k-chunks) + fp8 DR h-side (4 k-pairs)."""
    mo = bass.ts(m, P)
    for k in range(KX):
        for b in range(NBLK):
            nc.tensor.matmul(
                psl[b][:], wx_s[:, k, mo], xb_s[:, k, bsl[b]],
                start=(k == 0), stop=False,
            )
    for t in range(KH // 2):
        last = t == KH // 2 - 1
        for b in range(NBLK):
            nc.tensor.matmul(
                psl[b][:], wh_s[:, 2 * t : 2 * t + 2, mo],
                rh8_s[:, 2 * t : 2 * t + 2, bsl[b]],
                start=False, stop=last, perf_mode=DR,
            )


def _build():
    nc = bacc.Bacc("TRN2", target_bir_lowering=False, debug=False, num_devices=NCORES)

    a8 = nc.dram_tensor("a8", [P, KA, NB], F8, kind="ExternalInput").ap()
    hb = nc.dram_tensor("hb", [P, KH, NB], BF16, kind="ExternalInput").ap()
    xb = nc.dram_tensor("xb", [P, KX, NB], BF16, kind="ExternalInput").ap()
    wr = nc.dram_tensor("wr", [P, KA, HID], F8, kind="ExternalInput").ap()
    wz = nc.dram_tensor("wz", [P, KA, HID], F8, kind="ExternalInput").ap()
    wxh = nc.dram_tensor("wxh", [P, KX, HID], BF16, kind="ExternalInput").ap()
    whh = nc.dram_tensor("whh", [P, KH, HID], F8, kind="ExternalInput").ap()
    bias = nc.dram_tensor("bias", [P, 24], F32, kind="ExternalInput").ap()
    outT = nc.dram_tensor("outT", [P, M, NB], BF16, kind="ExternalOutput").ap()

    SIG = mybir.ActivationFunctionType.Sigmoid
    TANH = mybir.ActivationFunctionType.Tanh

    with tile.TileContext(nc) as tc:
        with (
            tc.tile_pool(name="wpool", bufs=1) as wpool,
            tc.tile_pool(name="actpool", bufs=1) as actpool,
            tc.tile_pool(name="rpool", bufs=3) as rpool,
            tc.tile_pool(name="hcpool", bufs=3) as hcpool,
            tc.tile_pool(name="opool", bufs=2) as opool,
            tc.tile_pool(name="psum", bufs=8, space=bass.MemorySpace.PSUM) as psum,
        ):
            bsl = [bass.ts(b, BLK) for b in range(NBLK)]

            # ---- resident tensors; DMA issue order = need order
            wr_s = wpool.tile([P, KA, HID], F8)
            nc.sync.dma_start(wr_s[:], wr[:])
            a8_s = actpool.tile([P, KA, NB], F8)
            for b in range(NBLK):
                nc.sync.dma_start(a8_s[:, :, bsl[b]], a8[:, :, bsl[b]])
            b_s = wpool.tile([P, 24], F32)
            nc.sync.dma_start(b_s[:], bias[:])
            hb_s = actpool.tile([P, KH, NB], BF16)
            for b in range(NBLK):
                nc.sync.dma_start(hb_s[:, :, bsl[b]], hb[:, :, bsl[b]])
            wz_s = wpool.tile([P, KA, HID], F8)
            nc.sync.dma_start(wz_s[:], wz[:])
            whh_s = wpool.tile([P, KH, HID], F8)
            nc.sync.dma_start(whh_s[:], whh[:])
            wxh_s = wpool.tile([P, KX, HID], BF16)
            nc.sync.dma_start(wxh_s[:], wxh[:])
            xb_s = actpool.tile([P, KX, NB], BF16)
            nc.sync.dma_start(xb_s[:], xb[:])

            rh8_s = actpool.tile([P, KH, NB], F8)
            zb_s = actpool.tile([P, M, NB], BF16)

            # ---- Phase R: r = sigmoid((x@Wxr + h@Whr)/SW + bxr); rh8 = f8(r*h)
            for m in range(M):
                psl = [psum.tile([P, BLK], F32, tag="ps", name="ps") for _ in range(NBLK)]
                _fp8_gate(nc, psl, wr_s, a8_s, m, bsl, blk_outer=(m == 0))
                for b in range(NBLK):
                    rt = rpool.tile([P, BLK], BF16, tag="rt")
                    nc.scalar.activation(rt[:], psl[b][:], SIG,
                                         bias=b_s[:, m : m + 1], scale=INV_SW)
                    nc.vector.tensor_mul(rh8_s[:, m, bsl[b]], rt[:], hb_s[:, m, bsl[b]])

            # ---- Phase Z: zb = bf16(sigmoid((x@Wxz + h@Whz)/SW + bxz))
            for m in range(M):
                psl = [psum.tile([P, BLK], F32, tag="ps", name="ps") for _ in range(NBLK)]
                _fp8_gate(nc, psl, wz_s, a8_s, m, bsl)
                for b in range(NBLK):
                    nc.scalar.activation(zb_s[:, m, bsl[b]], psl[b][:], SIG,
                                         bias=b_s[:, 8 + m : 9 + m], scale=INV_SW)

            # ---- Phase HC: hc = tanh((x@Wxh + rh@Whh)/SW + bxh); out = hc + z*(h-hc)
            for m in range(M):
                psl = [psum.tile([P, BLK], F32, tag="ps", name="ps") for _ in range(NBLK)]
                _hc_gate(nc, psl, wxh_s, xb_s, whh_s, rh8_s, m, bsl)
                last_m = m == M - 1
                ost = opool.tile([P, NB], BF16, tag="ost")
                for b in range(NBLK):
                    hct = hcpool.tile([P, BLK], BF16, tag="hct")
                    nc.scalar.activation(hct[:], psl[b][:], TANH,
                                         bias=b_s[:, 16 + m : 17 + m], scale=INV_SW)
                    ot = hcpool.tile([P, BLK], BF16, tag="ot")
                    nc.vector.tensor_sub(ot[:], hb_s[:, m, bsl[b]], hct[:])
                    nc.vector.tensor_mul(ot[:], ot[:], zb_s[:, m, bsl[b]])
                    nc.vector.tensor_add(ost[:, bsl[b]], ot[:], hct[:])
                    if last_m:
                        # drain the tail block-by-block so the final DMA is small
                        nc.sync.dma_start(outT[:, m, bsl[b]], ost[:, bsl[b]])
                if not last_m:
                    nc.sync.dma_start(outT[:, m, :], ost[:])

    nc.compile()
    return nc


def _pack_feature_major(a: np.ndarray, nchunks: int, dtype) -> np.ndarray:
    # [rows, cols] -> [128, nchunks, cols] with [p, k, c] = a[128k+p, c]
    rows, cols = a.shape
    assert rows == nchunks * P
    return np.ascontiguousarray(
        a.reshape(nchunks, P, cols).transpose(1, 0, 2)
    ).astype(dtype)


def _pack_inputs(x, hidden, Wxr, bxr, Whr, Wxz, bxz, Whz, Wxh, bxh, Whh):
    f8 = ml_dtypes.float8_e4m3
    bf = ml_dtypes.bfloat16

    def wcat(wx, wh):
        a = np.concatenate(
            [np.asarray(wx, np.float32) * SW, np.asarray(wh, np.float32) * SW], axis=0
        )
        return _pack_feature_major(a, KA, f8)

    common = {
        "wr": wcat(Wxr, Whr),
        "wz": wcat(Wxz, Whz),
        "wxh": _pack_feature_major(np.asarray(Wxh, np.float32) * SW, KX, bf),
        "whh": _pack_feature_major(np.asarray(Whh, np.float32) * SW, KH, f8),
        "bias": np.ascontiguousarray(
            np.concatenate(
                [np.asarray(b, np.float32).reshape(M, P).T for b in (bxr, bxz, bxh)],
                axis=1,
            )
        ),
    }

    x = np.asarray(x, np.float32)
    hidden = np.asarray(hidden, np.float32)
    in_maps = []
    for c in range(NCORES):
        rows = slice(c * NB, (c + 1) * NB)
        xT = x[rows].T
        hT = hidden[rows].T
        m = dict(common)
        m["a8"] = _pack_feature_major(np.concatenate([xT, hT], axis=0), KA, f8)
        m["xb"] = _pack_feature_major(xT, KX, bf)
        m["hb"] = _pack_feature_major(hT, KH, bf)
        in_maps.append(m)
    return in_maps


def kernel(x, hidden, Wxr, bxr, Whr, Wxz, bxz, Whz, Wxh, bxh, Whh):
    if "nc" not in _CACHE:
        _CACHE["nc"] = _build()
    nc = _CACHE["nc"]

    in_maps = _pack_inputs(x, hidden, Wxr, bxr, Whr, Wxz, bxz, Whz, Wxh, bxh, Whh)
    res = run_bass_kernel_spmd(nc, in_maps, core_ids=list(range(NCORES)))

    out = np.empty((BATCH, HID), np.float32)
    for c in range(NCORES):
        oT = np.asarray(res.results[c]["outT"], np.float32)  # [128, 8, 2048]
        out[c * NB : (c + 1) * NB] = oT.transpose(1, 0, 2).reshape(HID, NB).T
    return out


# revision 11
# speedup vs baseline: 1.0069x; 1.0069x over previous
"""GRU cell (single timestep) on 8 TRN2 NeuronCores, data-parallel over batch.

Contract: kernel(**inputs) takes FULL numpy inputs (as produced by the
problem's setup_inputs()) and returns the FULL (16384, 1024) float32 output.

Strategy (fp8 DoubleRow, v4):
  - Shard batch (16384) across 8 cores -> 2048 rows/core. Replicate weights.
  - Feature-major layout: contraction dim on partitions, batch on free dim.
      a8    [128, 12, 2048] f8  k-chunks 0-3 = x (e4m3), 4-11 = hidden (e4m3)
      hb    [128, 8, 2048] bf16 hidden for elementwise
      xb    [128, 4, 2048] bf16 x for the bf16 hc x-side matmul
      wr8   [128, 12, 1024] f8  Wxr (4 chunks) ++ Whr (8 chunks), pre-scaled
      wz8   [128, 12, 1024] f8  Wxz ++ Whz, pre-scaled
      wxh16 [128, 4, 1024] bf16 Wxh pre-scaled; whh8 [128, 8, 1024] f8
      bias  [128, 24] f32; outT [128, 8, 2048] bf16
  - r/z gates fully fp8 e4m3 with perf_mode=DoubleRow (2 contraction rows
    per PE cell per cycle); hc gate: x-side bf16 (precision-critical),
    h-side fp8 DR. All weights pre-scaled by 2048 (power of two); the
    activation instruction divides it back out. e4m3 operand error budget
    measured: rel_fro ~1.5e-2 vs the f32 reference (gate 2e-2).
  - Block loop (4 x 512 batch cols) INSIDE the k loop so one loaded
    stationary weight serves 4 matmuls (DoubleRow disables fast weight
    load; LDWEIGHTS must hide under 4 matmuls).
  - All post-matmul elementwise in bf16 (2x DVE); output written bf16 and
    upcast on host.
"""

import sys

if "/opt/trn_rl_repo" not in sys.path:
    sys.path.insert(0, "/opt/trn_rl_repo")

import numpy as np
import ml_dtypes

import concourse.bass as bass
import concourse.tile as tile
from concourse import bacc, mybir
from concourse.bass_utils import run_bass_kernel_spmd

P = 128
NCORES = 8
BATCH = 16384
NB = BATCH // NCORES          # 2048 rows per core
IN = 512
HID = 1024
KX = IN // P                  # 4
KH = HID // P                 # 8
KA = KX + KH                  # 12 chunks in the packed a8 tensor
M = HID // P                  # 8 output-feature chunks
BLK = 512                     # batch columns per block
NBLK = NB // BLK              # 4

F32 = mybir.dt.float32
BF16 = mybir.dt.bfloat16
F8 = mybir.dt.float8e4
DR = mybir.MatmulPerfMode.DoubleRow

SW = 2048.0                   # weight pre-scale (power of two, exact)
INV_SW = 1.0 / SW

_CACHE = {}


def _fp8_gate(nc, psl, w_s, act, m, bsl, blk_outer=False):
    """All-fp8 gate: 6 DoubleRow k-pairs over the packed 12-chunk operands."""
    mo = bass.ts(m, P)
    NT = KA // 2
    if blk_outer:
        # k-contiguous per block: used for the very first group so matmuls
        # can start as soon as block 0's operands arrive.
        for b in range(NBLK):
            for t in range(NT):
                nc.tensor.matmul(
                    psl[b][:], w_s[:, 2 * t : 2 * t + 2, mo],
                    act[:, 2 * t : 2 * t + 2, bsl[b]],
                    start=(t == 0), stop=(t == NT - 1), perf_mode=DR,
                )
    else:
        for t in range(NT):
            for b in range(NBLK):
                nc.tensor.matmul(
                    psl[b][:], w_s[:, 2 * t : 2 * t + 2, mo],
                    act[:, 2 * t : 2 * t + 2, bsl[b]],
                    start=(t == 0), stop=(t == NT - 1), perf_mode=DR,
                )


def _hc_gate(nc, psl, wx_s, xb_s, wh_s, rh8_s, m, bsl):
    """hc gate: bf16 x-side (4 k-chunks) + fp8 DR h-side (4 k-pairs)."""
    mo = bass.ts(m, P)
    for k in range(KX):
        for b in range(NBLK):
            nc.tensor.matmul(
                psl[b][:], wx_s[:, k, mo], xb_s[:, k, bsl[b]],
                start=(k == 0), stop=False,
            )
    for t in range(KH // 2):
        last = t == KH // 2 - 1
        for b in range(NBLK):
            nc.tensor.matmul(
                psl[b][:], wh_s[:, 2 * t : 2 * t + 2, mo],
                rh8_s[:, 2 * t : 2 * t + 2, bsl[b]],
                start=False, stop=last, perf_mode=DR,
            )


def _build():
    nc = bacc.Bacc("TRN2", target_bir_lowering=False, debug=False, num_devices=NCORES)

    a8 = nc.dram_tensor("a8", [P, KA, NB], F8, kind="ExternalInput").ap()
    hb = nc.dram_tensor("hb", [P, KH, NB], BF16, kind="ExternalInput").ap()
    xb = nc.dram_tensor("xb", [P, KX, NB], BF16, kind="ExternalInput").ap()
    wr = nc.dram_tensor("wr", [P, KA, HID], F8, kind="ExternalInput").ap()
    wz = nc.dram_tensor("wz", [P, KA, HID], F8, kind="ExternalInput").ap()
    wxh = nc.dram_tensor("wxh", [P, KX, HID], BF16, kind="ExternalInput").ap()
    whh = nc.dram_tensor("whh", [P, KH, HID], F8, kind="ExternalInput").ap()
    bias = nc.dram_tensor("bias", [P, 24], F32, kind="ExternalInput").ap()
    outT = nc.dram_tensor("outT", [P, M, NB], BF16, kind="ExternalOutput").ap()

    SIG = mybir.ActivationFunctionType.Sigmoid
    TANH = mybir.ActivationFunctionType.Tanh

    with tile.TileContext(nc) as tc:
        with (
            tc.tile_pool(name="wpool", bufs=1) as wpool,
            tc.tile_pool(name="actpool", bufs=1) as actpool,
            tc.tile_pool(name="rpool", bufs=3) as rpool,
            tc.tile_pool(name="hcpool", bufs=3) as hcpool,
            tc.tile_pool(name="opool", bufs=2) as opool,
            tc.tile_pool(name="psum", bufs=8, space=bass.MemorySpace.PSUM) as psum,
        ):
            bsl = [bass.ts(b, BLK) for b in range(NBLK)]

            # ---- resident tensors; DMA issue order = need order.
            # x/h parts split so the first matmuls (m=0, blk0, x-side) gate
            # on only wr[x-part] + a8[blk0, x-part] = 0.75 MB.
            wr_s = wpool.tile([P, KA, HID], F8)
            a8_s = actpool.tile([P, KA, NB], F8)
            nc.sync.dma_start(wr_s[:, :KX, :], wr[:, :KX, :])
            nc.sync.dma_start(a8_s[:, :KX, bsl[0]], a8[:, :KX, bsl[0]])
            nc.sync.dma_start(wr_s[:, KX:, :], wr[:, KX:, :])
            nc.sync.dma_start(a8_s[:, KX:, bsl[0]], a8[:, KX:, bsl[0]])
            for b in range(1, NBLK):
                nc.sync.dma_start(a8_s[:, :KX, bsl[b]], a8[:, :KX, bsl[b]])
                nc.sync.dma_start(a8_s[:, KX:, bsl[b]], a8[:, KX:, bsl[b]])
            b_s = wpool.tile([P, 24], F32)
            nc.sync.dma_start(b_s[:], bias[:])
            hb_s = actpool.tile([P, KH, NB], BF16)
            for b in range(NBLK):
                nc.sync.dma_start(hb_s[:, :, bsl[b]], hb[:, :, bsl[b]])
            wz_s = wpool.tile([P, KA, HID], F8)
            nc.sync.dma_start(wz_s[:], wz[:])
            whh_s = wpool.tile([P, KH, HID], F8)
            nc.sync.dma_start(whh_s[:], whh[:])
            wxh_s = wpool.tile([P, KX, HID], BF16)
            nc.sync.dma_start(wxh_s[:], wxh[:])
            xb_s = actpool.tile([P, KX, NB], BF16)
            nc.sync.dma_start(xb_s[:], xb[:])

            rh8_s = actpool.tile([P, KH, NB], F8)
            zb_s = actpool.tile([P, M, NB], BF16)

            # ---- Phase R: r = sigmoid((x@Wxr + h@Whr)/SW + bxr); rh8 = f8(r*h)
            for m in range(M):
                psl = [psum.tile([P, BLK], F32, tag="ps", name="ps") for _ in range(NBLK)]
                _fp8_gate(nc, psl, wr_s, a8_s, m, bsl, blk_outer=(m == 0))
                for b in range(NBLK):
                    rt = rpool.tile([P, BLK], BF16, tag="rt")
                    nc.scalar.activation(rt[:], psl[b][:], SIG,
                                         bias=b_s[:, m : m + 1], scale=INV_SW)
                    nc.vector.tensor_mul(rh8_s[:, m, bsl[b]], rt[:], hb_s[:, m, bsl[b]])

            # ---- Phase Z: zb = bf16(sigmoid((x@Wxz + h@Whz)/SW + bxz))
            for m in range(M):
                psl = [psum.tile([P, BLK], F32, tag="ps", name="ps") for _ in range(NBLK)]
                _fp8_gate(nc, psl, wz_s, a8_s, m, bsl)
                for b in range(NBLK):
                    nc.scalar.activation(zb_s[:, m, bsl[b]], psl[b][:], SIG,
                                         bias=b_s[:, 8 + m : 9 + m], scale=INV_SW)

            # ---- Phase HC: hc = tanh((x@Wxh + rh@Whh)/SW + bxh); out = hc + z*(h-hc)
            for m in range(M):
                psl = [psum.tile([P, BLK], F32, tag="ps", name="ps") for _ in range(NBLK)]
                _hc_gate(nc, psl, wxh_s, xb_s, whh_s, rh8_s, m, bsl)
                last_m = m == M - 1
                ost = opool.tile([P, NB], BF16, tag="ost")
                for b in range(NBLK):
                    hct = hcpool.tile([P, BLK], BF16, tag="hct")
                    nc.scalar.activation(hct[:], psl[b][:], TANH,
                                         bias=b_s[:, 16 + m : 17 + m], scale=INV_SW)
                    ot = hcpool.tile([P, BLK], BF16, tag="ot")
                    nc.vector.tensor_sub(ot[:], hb_s[:, m, bsl[b]], hct[:])
                    nc.vector.tensor_mul(ot[:], ot[:], zb_s[:, m, bsl[b]])
                    nc.vector.tensor_add(ost[:, bsl[b]], ot[:], hct[:])
                    if last_m:
                        # drain the tail block-by-block so the final DMA is small
                        nc.sync.dma_start(outT[:, m, bsl[b]], ost[:, bsl[b]])
                if not last_m:
                    nc.sync.dma_start(outT[:, m, :], ost[:])

    nc.compile()
    return nc


def _pack_feature_major(a: np.ndarray, nchunks: int, dtype) -> np.ndarray:
    # [rows, cols] -> [128, nchunks, cols] with [p, k, c] = a[128k+p, c]
    rows, cols = a.shape
    assert rows == nchunks * P
    return np.ascontiguousarray(
        a.reshape(nchunks, P, cols).transpose(1, 0, 2)
    ).astype(dtype)


def _pack_inputs(x, hidden, Wxr, bxr, Whr, Wxz, bxz, Whz, Wxh, bxh, Whh):
    f8 = ml_dtypes.float8_e4m3
    bf = ml_dtypes.bfloat16

    def wcat(wx, wh):
        a = np.concatenate(
            [np.asarray(wx, np.float32) * SW, np.asarray(wh, np.float32) * SW], axis=0
        )
        return _pack_feature_major(a, KA, f8)

    common = {
        "wr": wcat(Wxr, Whr),
        "wz": wcat(Wxz, Whz),
        "wxh": _pack_feature_major(np.asarray(Wxh, np.float32) * SW, KX, bf),
        "whh": _pack_feature_major(np.asarray(Whh, np.float32) * SW, KH, f8),
        "bias": np.ascontiguousarray(
            np.concatenate(
                [np.asarray(b, np.float32).reshape(M, P).T for b in (bxr, bxz, bxh)],
                axis=1,
            )
        ),
    }

    x = np.asarray(x, np.float32)
    hidden = np.asarray(hidden, np.float32)
    in_maps = []
    for c in range(NCORES):
        rows = slice(c * NB, (c + 1) * NB)
        xT = x[rows].T
        hT = hidden[rows].T
        m = dict(common)
        m["a8"] = _pack_feature_major(np.concatenate([xT, hT], axis=0), KA, f8)
        m["xb"] = _pack_feature_major(xT, KX, bf)
        m["hb"] = _pack_feature_major(hT, KH, bf)
        in_maps.append(m)
    return in_maps


def kernel(x, hidden, Wxr, bxr, Whr, Wxz, bxz, Whz, Wxh, bxh, Whh):
    if "nc" not in _CACHE:
        _CACHE["nc"] = _build()
    nc = _CACHE["nc"]

    in_maps = _pack_inputs(x, hidden, Wxr, bxr, Whr, Wxz, bxz, Whz, Wxh, bxh, Whh)
    res = run_bass_kernel_spmd(nc, in_maps, core_ids=list(range(NCORES)))

    out = np.empty((BATCH, HID), np.float32)
    for c in range(NCORES):
        oT = np.asarray(res.results[c]["outT"], np.float32)  # [128, 8, 2048]
        out[c * NB : (c + 1) * NB] = oT.transpose(1, 0, 2).reshape(HID, NB).T
    return out
